# revision 1
# baseline (speedup 1.0000x reference)
"""Trainium2 Bass kernel for per-token head-attention transformer block.

Reference computation (N=16, T=4096, D=1024, H=16, hd=64):
    qkv = x @ w_qkv + b_qkv                       (N,T,3D)
    q,k,v = split(qkv)  each (N,T,H,hd)
    S = einsum('nthd,ntgd->nthg', q*hd^-0.5, k)   per-token 16x16 over heads
    P = softmax(S, -1)
    o = einsum('nthg,ntgd->nthd', P, v)
    out = o.transpose(0,2,1,3).reshape(N,T,D) @ w_proj + b_proj

Mapping: data-parallel over batch N across 8 cores (2 batch elements each).
Per core:
  - qkv / proj matmuls on the PE in float32r (full-rate, fp22 mantissa).
  - activations flow feature-major (PE transposes); per-token head-attention
    runs on DVE with broadcast/strided access patterns in bf16.
  - attention output is spilled to DRAM head-major (H,T,hd): reinterpreting
    that buffer as a (T,D) row-major matrix IS the reference's
    transpose(0,2,1,3).reshape -- the proj matmul just reads it back.
"""

import sys

sys.path.insert(0, "/opt/trn_rl_repo")

from contextlib import ExitStack

import numpy as np

import concourse.bass as bass
import concourse.tile as tile
from concourse import mybir
from concourse.bass_utils import run_bass_kernel_spmd
from concourse.masks import make_identity

N, T, D = 16, 4096, 1024
H, HD = 16, 64
NCORES = 8
NB = N // NCORES  # batch elements per core
SCALE = float(HD) ** -0.5

F32 = mybir.dt.float32
F32R = mybir.dt.float32r
BF16 = mybir.dt.bfloat16
ATT_DT = BF16  # dtype of q/k/v/P and the attention products

SKIP_ATT = False   # debug: skip attention math
SKIP_TOKT = False  # debug: skip feature->token transposes
SKIP_P2 = False    # debug: skip proj phase
CH = 256          # token chunk (matmul moving dim; must be >=256 for f32r rate)
NT = CH // 128    # token tiles per chunk
KD = D // 128     # contraction chunks (8)
JQ = 3 * D // 128  # qkv output feature chunks (24)
JP = D // 128     # proj output feature chunks (8)
NCH = T // CH     # chunks per batch element (16)
HG = 2            # heads per attention product group
NHG = H // HG     # number of product groups (8)

Ident = mybir.ActivationFunctionType.Identity
Exp = mybir.ActivationFunctionType.Exp
ALU = mybir.AluOpType
AX = mybir.AxisListType


def _ap(sl, dims):
    """Custom free-dim access pattern on a sliced tile: keep partition dim +
    offset of `sl`, replace free dims with [step, num] list `dims`."""
    return bass.AP(tensor=sl.tensor, offset=sl.offset, ap=[sl.ap[0]] + dims)


def build_kernel():
    nc = bass.Bass()
    x = nc.dram_tensor("x", [NB * T, D], F32, kind="ExternalInput")
    wqkv = nc.dram_tensor("w_qkv", [D, 3 * D], F32, kind="ExternalInput")
    bqkv = nc.dram_tensor("b_qkv", [3 * D], F32, kind="ExternalInput")
    wproj = nc.dram_tensor("w_proj", [D, D], F32, kind="ExternalInput")
    bproj = nc.dram_tensor("b_proj", [D], F32, kind="ExternalInput")
    y = nc.dram_tensor("y", [NB * T, D], F32, kind="ExternalOutput")

    with ExitStack() as ctx:
        tc = ctx.enter_context(tile.TileContext(nc))
        singles = ctx.enter_context(tc.tile_pool(name="singles", bufs=1))
        xp = ctx.enter_context(tc.tile_pool(name="xp", bufs=2))
        xtp = ctx.enter_context(tc.tile_pool(name="xtp", bufs=1))
        ytp = ctx.enter_context(tc.tile_pool(name="ytp", bufs=1))
        qkvp = ctx.enter_context(tc.tile_pool(name="qkvp", bufs=1))
        tokp = ctx.enter_context(tc.tile_pool(name="tokp", bufs=2))
        att = ctx.enter_context(tc.tile_pool(name="att", bufs=2))
        outp = ctx.enter_context(tc.tile_pool(name="outp", bufs=2))
        prodp = ctx.enter_context(tc.tile_pool(name="prodp", bufs=2))
        vtp = ctx.enter_context(tc.tile_pool(name="vtp", bufs=1))
        psA = ctx.enter_context(tc.tile_pool(name="psA", bufs=4, space="PSUM"))
        psT = ctx.enter_context(tc.tile_pool(name="psT", bufs=4, space="PSUM"))
        dram = ctx.enter_context(tc.tile_pool(name="dram", bufs=1, space="DRAM"))

        ident = singles.tile([128, 128], F32)
        make_identity(nc, ident)
        ident_b = singles.tile([128, 128], ATT_DT)
        make_identity(nc, ident_b)

        # resident weights, (in,out) layout chunked over contraction dim
        wq_s = singles.tile([128, KD, 3 * D], F32R)
        wq_src = wqkv.rearrange("(k p) j -> p k j", p=128).bitcast(F32R)
        for k in range(KD):
            nc.gpsimd.dma_start(out=wq_s[:, k, :], in_=wq_src[:, k, :])
        wp_s = singles.tile([128, KD, D], F32R)
        wp_src = wproj.rearrange("(k p) j -> p k j", p=128).bitcast(F32R)
        for k in range(KD):
            nc.gpsimd.dma_start(out=wp_s[:, k, :], in_=wp_src[:, k, :])
        # biases as [128, n_chunks] columns
        bq_s = singles.tile([128, JQ], F32)
        nc.gpsimd.dma_start(out=bq_s, in_=bqkv.rearrange("(j p) -> p j", p=128))
        bp_s = singles.tile([128, JP], F32)
        nc.gpsimd.dma_start(out=bp_s, in_=bproj.rearrange("(j p) -> p j", p=128))
        # pre-scaled q biases: q path computes SCALE*(x@Wq) + SCALE*bq
        bq_sc = singles.tile([128, JQ // 3], F32)
        nc.scalar.mul(bq_sc, bq_s[:, 0 : JQ // 3], SCALE)

        # head-major attention-output spill: flat layout h*(T*HD) + t*HD + d,
        # viewed by phase 2 as a row-major (T, D) matrix per batch element.
        aspill = dram.tile([NB, T, D], F32)

        def p1_chunk(n, c):
            t0 = c * CH
            xT = xtp.tile([128, KD, CH], F32R, tag="xT")
            for tt in range(NT):
                xt = xp.tile([128, D], F32, tag="x")
                r0 = n * T + t0 + tt * 128
                nc.sync.dma_start(out=xt, in_=x[r0 : r0 + 128, :])
                for k in range(KD):
                    pt = psT.tile([128, 128], F32, tag="tp")
                    nc.tensor.transpose(pt, xt[:, k * 128 : (k + 1) * 128], ident)
                    nc.scalar.copy(
                        out=xT[:, k, tt * 128 : (tt + 1) * 128], in_=pt
                    )

            qkvT = qkvp.tile([128, JQ, CH], ATT_DT, tag="qkvT")
            for j in range(JQ):
                pm = psA.tile([128, CH], F32, tag="mm")
                for k in range(KD):
                    nc.tensor.matmul(
                        pm,
                        wq_s[:, k, j * 128 : (j + 1) * 128],
                        xT[:, k, :],
                        start=(k == 0),
                        stop=(k == KD - 1),
                    )
                if j < JQ // 3:  # q: fold in attention scale
                    nc.scalar.activation(
                        out=qkvT[:, j, :], in_=pm, func=Ident,
                        bias=bq_sc[:, j : j + 1], scale=SCALE,
                    )
                else:
                    nc.scalar.activation(
                        out=qkvT[:, j, :], in_=pm, func=Ident,
                        bias=bq_s[:, j : j + 1], scale=1.0,
                    )

            for tt in range(NT):
                # feature-major -> token-major for the per-token attention
                tok = tokp.tile([128, 3 * D], ATT_DT, tag="tok")
                for j in range(0 if not SKIP_TOKT else JQ, JQ):
                    pt = psT.tile([128, 128], ATT_DT, tag="tp")
                    nc.tensor.transpose(
                        pt, qkvT[:, j, tt * 128 : (tt + 1) * 128], ident_b
                    )
                    nc.scalar.copy(out=tok[:, j * 128 : (j + 1) * 128], in_=pt)

                # v in (d, g) layout so the AV product's innermost dim
                # (g) is packed in BOTH operands -> DVE 2x mode
                vt = vtp.tile([128, HD, H], ATT_DT, tag="vt")
                vsrc = tok[:, 2 * D : 3 * D]
                nc.gpsimd.tensor_copy(
                    out=vt, in_=_ap(vsrc, [[1, HD], [HD, H]])
                )
                # scores S[t,h,g] = sum_d q[t,h,d] k[t,g,d] (q pre-scaled)
                if SKIP_ATT:
                    O = outp.tile([128, H, HD], F32, tag="O")
                    nc.scalar.copy(out=O.rearrange("p h d -> p (h d)"), in_=tok[:, :D])
                    base = aspill[n]
                    dst = bass.AP(
                        tensor=base.tensor,
                        offset=base.offset + (t0 + tt * 128) * HD,
                        ap=[[HD, 128], [T * HD, H], [1, HD]],
                    )
                    nc.sync.dma_start(out=dst, in_=O)
                    continue
                S = att.tile([128, H, H], F32, tag="S")
                for g0 in range(NHG):
                    prod = prodp.tile([128, HG, H, HD], ATT_DT, tag="prod")
                    qs = tok[:, g0 * HG * HD : (g0 * HG + HG) * HD]
                    ks = tok[:, D : 2 * D]
                    nc.vector.tensor_tensor(
                        out=prod,
                        in0=_ap(qs, [[HD, HG], [0, H], [1, HD]]),
                        in1=_ap(ks, [[0, HG], [HD, H], [1, HD]]),
                        op=ALU.mult,
                    )
                    half = prodp.tile([128, HG, H, HD // 2], ATT_DT, tag="half")
                    nc.vector.tensor_tensor(
                        out=half,
                        in0=prod[:, :, :, 0 : HD // 2],
                        in1=prod[:, :, :, HD // 2 : HD],
                        op=ALU.add,
                    )
                    quar = prodp.tile([128, HG, H, HD // 4], ATT_DT, tag="half")
                    nc.vector.tensor_tensor(
                        out=quar,
                        in0=half[:, :, :, 0 : HD // 4],
                        in1=half[:, :, :, HD // 4 : HD // 2],
                        op=ALU.add,
                    )
                    nc.vector.tensor_reduce(
                        out=S[:, g0 * HG : (g0 + 1) * HG, :],
                        in_=quar, axis=AX.X, op=ALU.add,
                    )

                # softmax over g
                mx = att.tile([128, H], F32, tag="mx")
                nc.vector.tensor_reduce(out=mx, in_=S, axis=AX.X, op=ALU.max)
                nc.vector.tensor_tensor(
                    out=S, in0=S, in1=_ap(mx[:, :], [[1, H], [0, H]]),
                    op=ALU.subtract,
                )
                nc.scalar.activation(out=S, in_=S, func=Exp)
                sm = att.tile([128, H], F32, tag="sm")
                nc.vector.tensor_reduce(out=sm, in_=S, axis=AX.X, op=ALU.add)
                rs = att.tile([128, H], F32, tag="rs")
                nc.vector.reciprocal(rs, sm)
                P = att.tile([128, H, H], ATT_DT, tag="P")
                nc.vector.tensor_tensor(
                    out=P, in0=S, in1=_ap(rs[:, :], [[1, H], [0, H]]),
                    op=ALU.mult,
                )

                # o[t,h,d] = sum_g P[t,h,g] v[t,g,d]
                O = outp.tile([128, H, HD], F32, tag="O")
                for g0 in range(NHG):
                    prod2 = prodp.tile([128, HG, HD, H], ATT_DT, tag="prod")
                    ps = P[:, g0 * HG, :]
                    nc.vector.tensor_tensor(
                        out=prod2,
                        in0=_ap(ps, [[H, HG], [0, HD], [1, H]]),
                        in1=_ap(vt[:, :], [[0, HG], [H, HD], [1, H]]),
                        op=ALU.mult,
                    )
                    half2 = prodp.tile([128, HG, HD, H // 2], ATT_DT, tag="half")
                    nc.vector.tensor_tensor(
                        out=half2,
                        in0=prod2[:, :, :, 0 : H // 2],
                        in1=prod2[:, :, :, H // 2 : H],
                        op=ALU.add,
                    )
                    quar2 = prodp.tile([128, HG, HD, H // 4], ATT_DT, tag="half")
                    nc.vector.tensor_tensor(
                        out=quar2,
                        in0=half2[:, :, :, 0 : H // 4],
                        in1=half2[:, :, :, H // 4 : H // 2],
                        op=ALU.add,
                    )
                    nc.vector.tensor_reduce(
                        out=O[:, g0 * HG : (g0 + 1) * HG, :],
                        in_=quar2, axis=AX.X, op=ALU.add,
                    )

                # spill head-major: dst[h, t, d] = O[t, h, d]; two half
                # spills so the first streams while the second half computes
                base = aspill[n]  # flat (T*D) region, offset n*T*D
                for h0 in range(0, H, H // 4):
                    dst = bass.AP(
                        tensor=base.tensor,
                        offset=base.offset + h0 * T * HD + (t0 + tt * 128) * HD,
                        ap=[[HD, 128], [T * HD, H // 4], [1, HD]],
                    )
                    nc.sync.dma_start(out=dst, in_=O[:, h0 : h0 + H // 4, :])

        def p2_chunk(n, c):
            t0 = c * CH
            AT = xtp.tile([128, KD, CH], F32R, tag="xT")
            for tt in range(NT):
                at = xp.tile([128, D], F32, tag="x")
                nc.sync.dma_start(
                    out=at, in_=aspill[n, t0 + tt * 128 : t0 + tt * 128 + 128, :]
                )
                for k in range(KD):
                    pt = psT.tile([128, 128], F32, tag="tp")
                    nc.tensor.transpose(pt, at[:, k * 128 : (k + 1) * 128], ident)
                    nc.scalar.copy(
                        out=AT[:, k, tt * 128 : (tt + 1) * 128], in_=pt
                    )
            yT = ytp.tile([128, JP, CH], F32, tag="yT")
            for j in range(JP):
                pm = psA.tile([128, CH], F32, tag="mm")
                for k in range(KD):
                    nc.tensor.matmul(
                        pm,
                        wp_s[:, k, j * 128 : (j + 1) * 128],
                        AT[:, k, :],
                        start=(k == 0),
                        stop=(k == KD - 1),
                    )
                nc.scalar.activation(
                    out=yT[:, j, :], in_=pm, func=Ident,
                    bias=bp_s[:, j : j + 1], scale=1.0,
                )
            for tt in range(NT):
                yt = ytp.tile([128, D], F32, tag="y")
                for j in range(JP):
                    pt = psT.tile([128, 128], F32, tag="tp")
                    nc.tensor.transpose(
                        pt, yT[:, j, tt * 128 : (tt + 1) * 128], ident
                    )
                    nc.vector.tensor_copy(
                        out=yt[:, j * 128 : (j + 1) * 128], in_=pt
                    )
                r0 = n * T + t0 + tt * 128
                nc.sync.dma_start(out=y[r0 : r0 + 128, :], in_=yt)

        # interleave: phase-2 of batch element n (PE/ACT-heavy) overlaps
        # phase-1 of n+1 (DVE-heavy attention)
        for c in range(NCH):
            p1_chunk(0, c)
        for n in range(1, NB):
            for c in range(NCH):
                if not SKIP_P2:
                    p2_chunk(n - 1, c)
                p1_chunk(n, c)
        if not SKIP_P2:
            for c in range(NCH):
                p2_chunk(NB - 1, c)

    # TRN2 allows at most one sync wait per engine instruction; split
    # multi-wait instructions through event semaphores.
    import bass_rust

    bass_rust.generate_event_semaphores(nc)
    return nc


_NC_CACHE = None
TRACE = False
LAST_RESULTS = None


def kernel(x, w_qkv, b_qkv, w_proj, b_proj):
    global _NC_CACHE, LAST_RESULTS
    if _NC_CACHE is None:
        _NC_CACHE = build_kernel()
    nc = _NC_CACHE
    x = np.ascontiguousarray(np.asarray(x, dtype=np.float32))
    w_qkv = np.ascontiguousarray(np.asarray(w_qkv, dtype=np.float32))
    b_qkv = np.ascontiguousarray(np.asarray(b_qkv, dtype=np.float32))
    w_proj = np.ascontiguousarray(np.asarray(w_proj, dtype=np.float32))
    b_proj = np.ascontiguousarray(np.asarray(b_proj, dtype=np.float32))
    in_maps = []
    for i in range(NCORES):
        in_maps.append(
            {
                "x": x[i * NB : (i + 1) * NB].reshape(NB * T, D),
                "w_qkv": w_qkv,
                "b_qkv": b_qkv,
                "w_proj": w_proj,
                "b_proj": b_proj,
            }
        )
    res = run_bass_kernel_spmd(
        nc, in_maps, core_ids=list(range(NCORES)), trace=TRACE
    )
    LAST_RESULTS = res
    out = np.empty((N, T, D), dtype=np.float32)
    for i in range(NCORES):
        out[i * NB : (i + 1) * NB] = res.results[i]["y"].reshape(NB, T, D)
    return out


if __name__ == "__main__":
    rng = np.random.default_rng(0)
    inputs = {
        "x": rng.standard_normal((N, T, D), dtype=np.float32),
        "w_qkv": rng.standard_normal((D, 3 * D), dtype=np.float32) * D**-0.5,
        "b_qkv": rng.standard_normal((3 * D,), dtype=np.float32) * 0.02,
        "w_proj": rng.standard_normal((D, D), dtype=np.float32) * D**-0.5,
        "b_proj": rng.standard_normal((D,), dtype=np.float32) * 0.02,
    }
    out = kernel(**inputs)
    print("out", out.shape, out.dtype)

